# revision 6
# baseline (speedup 1.0000x reference)
"""Trainium2 Bass kernel for nn_DSEBlock: FEA (multi-scale bilinear edge) +
DoG (difference-of-gaussians depthwise) + 1x1 mixer, data-parallel over batch
on 8 NeuronCores.

Decomposition (validated vs reference to ~1e-6):
  y = dec + skip
  per scale s in {.25,.5,.75}:  r_s = (Uh Dh) x (Uw Dw) y  (separable bilinear
      down+up);  d_s = y - r_s
  w_edge = (2/3)(max_s|d_s| - min_s|d_s|)        [mean abs pairwise diff of 3]
  dog    = n1*G1 - n2*G2, G_i = sep conv with unnormalized [t,1,t] x [t,1,t]
  z      = 3y + w_fea*w_edge + dog;  out = mixer @ z + skip

Layouts: back half in C-layout [c(128), h, w]; FEA front in W-layout
[w(96), h, c] reached via PE transposes.  W-axis resize runs on PE as a 96x96
matrix; H-axis resize runs on DVE as per-residue strided lerp ops fused with
the final subtract.
"""
import functools
import math

import ml_dtypes
import numpy as np

import concourse.bass as bass
import concourse.mybir as mybir
import concourse.tile as tile
from concourse import bacc
from concourse.bass import ts
from concourse.bass_utils import run_bass_kernel_spmd
from concourse.masks import make_identity

F32 = mybir.dt.float32
BF16 = mybir.dt.bfloat16
AL = mybir.AluOpType
AF = mybir.ActivationFunctionType

B, C, H, W = 16, 256, 96, 96
NCORES = 8
BPC = B // NCORES  # samples per core
SCALES = [0.25, 0.5, 0.75]
NS = [24, 48, 72]
HW = H * W


def _sl(start, step, cnt):
    return slice(start, start + step * (cnt - 1) + 1, step)


# ---------------- host-side resize specs ----------------
def _resize_matrix(n_in, n_out):
    R = np.zeros((n_out, n_in), dtype=np.float64)
    scale = n_in / n_out
    for j in range(n_out):
        x = (j + 0.5) * scale - 0.5
        i0 = int(np.floor(x))
        f = x - i0
        R[j, min(max(i0, 0), n_in - 1)] += 1.0 - f
        R[j, min(max(i0 + 1, 0), n_in - 1)] += f
    return R


def _down_ops(s):
    # ("avg", (o0,ostep,cnt), (a0,astep), (b0,bstep))  -> out = a+b (x0.5 folded
    #   into the PE W-matrix)
    # ("lerp", (o0,ostep,cnt), (a0,astep), (b0,bstep), f) -> out=(1-f)a+f*b
    if s == 0.25:
        return [("avg", (0, 1, 24), (1, 4), (2, 4))]
    if s == 0.5:
        return [("avg", (0, 1, 48), (0, 2), (1, 2))]
    assert s == 0.75
    return [
        ("lerp", (r, 3, 24), (i0, 4), (i0 + 1, 4), f)
        for r, (i0, f) in enumerate([(0, 1 / 6), (1, 1 / 2), (2, 5 / 6)])
    ]


def _up_ops(s):
    ns = int(H * s)
    scale = ns / H
    S = {0.25: 1, 0.5: 2, 0.75: 3}[s]
    groups, copies = {}, []
    for j in range(H):
        x = (j + 0.5) * scale - 0.5
        i0 = int(np.floor(x))
        f = x - i0
        if i0 < 0:
            copies.append((j, 0))
            continue
        if i0 + 1 > ns - 1:
            copies.append((j, ns - 1))
            continue
        r, m = j % 4, j // 4
        groups.setdefault((r, i0 - S * m, round(f * 48)), []).append((m, i0, f))
    res_ops = []
    for (r, o, _), items in groups.items():
        ms = sorted(m for m, _, _ in items)
        assert ms == list(range(ms[0], ms[-1] + 1))
        f = items[0][2]
        a0 = S * ms[0] + o
        res_ops.append(((4 * ms[0] + r, 4, len(ms)), (a0, S), (a0 + 1, S), f))
    return res_ops, copies


# ---------------- program (input-independent; cached) ----------------
@functools.lru_cache(maxsize=1)
def _build():
    nc = bacc.Bacc("TRN2", target_bir_lowering=False, debug=False)
    dec_d = nc.dram_tensor("dec", [BPC, C, H, W], F32, kind="ExternalInput")
    skip_d = nc.dram_tensor("skip", [BPC, C, H, W], F32, kind="ExternalInput")
    aw_d = nc.dram_tensor("aw", [96, 3, 96], BF16, kind="ExternalInput")
    mw_d = nc.dram_tensor("mw", [128, 4, 128], BF16, kind="ExternalInput")
    coef_d = nc.dram_tensor("coef", [128, 12], F32, kind="ExternalInput")
    out_d = nc.dram_tensor("out", [BPC, C, H, W], F32, kind="ExternalOutput")

    dn_ops = [_down_ops(s) for s in SCALES]
    up_res = [_up_ops(s)[0] for s in SCALES]
    up_cp = [_up_ops(s)[1] for s in SCALES]

    with tile.TileContext(nc) as tc:
        with (
            tc.tile_pool(name="const", bufs=1) as pconst,
            tc.tile_pool(name="stage", bufs=2) as pstage,
            tc.tile_pool(name="py", bufs=1) as py,
            tc.tile_pool(name="pyw", bufs=1) as pyw,
            tc.tile_pool(name="pfr", bufs=1) as pfr,      # mx/mn/d per-tag
            tc.tile_pool(name="psm", bufs=1) as psm,      # hd/t2
            tc.tile_pool(name="ptmp", bufs=2) as ptmp,    # small temps
            tc.tile_pool(name="pwc", bufs=1) as pwc,
            tc.tile_pool(name="pdog", bufs=2) as pdog,
            tc.tile_pool(name="pp4", bufs=2) as pp4,
            tc.tile_pool(name="ps_ytr", bufs=2, space="PSUM") as ps_ytr,
            tc.tile_pool(name="ps_mm", bufs=2, space="PSUM") as ps_mm,
            tc.tile_pool(name="ps_wed", bufs=2, space="PSUM") as ps_wed,
            tc.tile_pool(name="ps_mix", bufs=2, space="PSUM") as ps_mix,
            tc.tile_pool(name="dram", bufs=2, space="DRAM") as pdram,
        ):
            # constants
            aw_sb = pconst.tile([96, 3, 96], BF16)
            nc.sync.dma_start(out=aw_sb[:], in_=aw_d[:])
            mw_sb = pconst.tile([128, 4, 128], BF16)
            nc.sync.dma_start(out=mw_sb[:], in_=mw_d[:])
            coef_sb = pconst.tile([128, 12], F32)
            nc.sync.dma_start(out=coef_sb[:], in_=coef_d[:])
            ident = pconst.tile([128, 128], BF16)
            make_identity(nc, ident[:])

            def cf(cb, j, psl=slice(0, 128)):
                return coef_sb[psl, cb * 6 + j : cb * 6 + j + 1]

            for s in range(BPC):
                zds = [
                    pdram.tile([128, HW], BF16, tag=f"zd{cb}", name=f"zd{cb}_{s}")
                    for cb in range(2)
                ]
                for cb in range(2):
                    csl_d = slice(cb * 128, (cb + 1) * 128)
                    # ---- P1: y = dec + skip (bf16), 12-row strips ----
                    y = py.tile([128, H, W], BF16, tag="y", name=f"y_{s}_{cb}")
                    for st in range(8):
                        rsl = slice(st * 12, st * 12 + 12)
                        td = pstage.tile([128, 12, W], F32, tag="std", name=f"td{s}{cb}{st}")
                        nc.sync.dma_start(out=td[:], in_=dec_d[s, csl_d, rsl])
                        tk = pstage.tile([128, 12, W], F32, tag="stk", name=f"tk{s}{cb}{st}")
                        nc.sync.dma_start(out=tk[:], in_=skip_d[s, csl_d, rsl])
                        nc.vector.tensor_add(out=y[:, rsl, :], in0=td[:], in1=tk[:])

                    # ---- P2: FEA front ----
                    yw = pyw.tile([96, H, 128], BF16, tag="yw", name=f"yw_{s}_{cb}")
                    for hb in range(24):
                        pt = ps_ytr.tile([96, 4, 128], BF16, tag="ptr", name=f"ptr{s}{cb}{hb}")
                        for k in range(4):
                            nc.tensor.transpose(pt[:, k, :], y[:, hb * 4 + k, :], ident[:])
                        nc.scalar.copy(yw[:, hb * 4 : hb * 4 + 4, :], pt[:])

                    wcon = pwc.tile([128, H, W], BF16, tag="wcon", name=f"wc_{s}_{cb}")
                    for hf in range(2):
                        cfs = slice(hf * 64, hf * 64 + 64)  # free c slice in W-layout
                        cps = slice(hf * 64, hf * 64 + 64)  # partition slice in C-layout
                        mx = pfr.tile([96, H, 64], BF16, tag="mx", name=f"mx{s}{cb}{hf}")
                        mn = pfr.tile([96, H, 64], BF16, tag="mn", name=f"mn{s}{cb}{hf}")
                        for si in range(3):
                            ns = NS[si]
                            # H-down (free axis h)
                            hd = psm.tile([96, 72, 64], BF16, tag="hd", name=f"hd{s}{cb}{hf}{si}")
                            for op in dn_ops[si]:
                                if op[0] == "avg":
                                    (o0, ostep, cnt), (a0, astep), (b0, bstep) = op[1:]
                                    nc.vector.tensor_add(
                                        out=hd[:, _sl(o0, ostep, cnt), :],
                                        in0=yw[:, _sl(a0, astep, cnt), cfs],
                                        in1=yw[:, _sl(b0, bstep, cnt), cfs],
                                    )
                                else:
                                    (o0, ostep, cnt), (a0, astep), (b0, bstep), f = op[1:]
                                    dl = ptmp.tile([96, 24, 64], BF16, tag="dl")
                                    nc.vector.tensor_sub(
                                        out=dl[:],
                                        in0=yw[:, _sl(b0, bstep, cnt), cfs],
                                        in1=yw[:, _sl(a0, astep, cnt), cfs],
                                    )
                                    nc.vector.scalar_tensor_tensor(
                                        out=hd[:, _sl(o0, ostep, cnt), :],
                                        in0=dl[:],
                                        scalar=float(f),
                                        in1=yw[:, _sl(a0, astep, cnt), cfs],
                                        op0=AL.mult,
                                        op1=AL.add,
                                    )
                            # W transform on PE
                            t2 = psm.tile([96, 72, 64], BF16, tag="t2", name=f"t2{s}{cb}{hf}{si}")
                            hdf = hd[:, :ns, :].rearrange("w h c -> w (h c)")
                            t2f = t2[:, :ns, :].rearrange("w h c -> w (h c)")
                            for ntm in range(ns * 64 // 512):
                                pm = ps_mm.tile([96, 512], F32, tag="pmm")
                                nc.tensor.matmul(
                                    pm[:],
                                    lhsT=aw_sb[:, si, :],
                                    rhs=hdf[:, ts(ntm, 512)],
                                    start=True,
                                    stop=True,
                                )
                                nc.scalar.copy(t2f[:, ts(ntm, 512)], pm[:])
                            # H-up fused with subtract: d = y - up(t2)
                            d = pfr.tile([96, H, 64], BF16, tag="d", name=f"d{s}{cb}{hf}{si}")
                            for (o0, _, ocnt), (a0, S0), (b0, S1), f in up_res[si]:
                                ut = ptmp.tile([96, 24, 64], BF16, tag="ut")
                                nc.vector.scalar_tensor_tensor(
                                    out=ut[:, :ocnt, :],
                                    in0=t2[:, _sl(a0, S0, ocnt), :],
                                    scalar=float(-(1.0 - f)),
                                    in1=yw[:, _sl(o0, 4, ocnt), cfs],
                                    op0=AL.mult,
                                    op1=AL.add,
                                )
                                nc.vector.scalar_tensor_tensor(
                                    out=d[:, _sl(o0, 4, ocnt), :],
                                    in0=t2[:, _sl(b0, S1, ocnt), :],
                                    scalar=float(-f),
                                    in1=ut[:, :ocnt, :],
                                    op0=AL.mult,
                                    op1=AL.add,
                                )
                            for j, src in up_cp[si]:
                                nc.vector.tensor_sub(
                                    out=d[:, j, :], in0=yw[:, j, cfs], in1=t2[:, src, :]
                                )
                            # running max/min of |d|
                            if si == 0:
                                nc.scalar.activation(mx[:], d[:], AF.Abs)
                                nc.scalar.activation(mn[:], d[:], AF.Abs)
                            else:
                                nc.scalar.activation(d[:], d[:], AF.Abs)
                                nc.vector.tensor_tensor(
                                    out=mx[:], in0=mx[:], in1=d[:], op=AL.max
                                )
                                nc.vector.tensor_tensor(
                                    out=mn[:], in0=mn[:], in1=d[:], op=AL.min
                                )
                        # wedge = mx - mn (in place), transpose back, scale w_fea'
                        nc.vector.tensor_sub(out=mx[:], in0=mx[:], in1=mn[:])
                        for hb in range(24):
                            pw = ps_wed.tile([128, 4, 96], BF16, tag="pwed")
                            for k in range(4):
                                nc.tensor.transpose(
                                    pw[cps, k, :], mx[:, hb * 4 + k, :], ident[0:96, 0:96]
                                )
                            nc.scalar.activation(
                                wcon[cps, hb * 4 : hb * 4 + 4, :],
                                pw[cps],
                                AF.Copy,
                                scale=cf(cb, 0, cps),
                            )

                    # ---- P3: DoG + z accumulation, 12-row strips ----
                    for st in range(8):
                        h0 = st * 12
                        r0, r1 = max(h0 - 1, 0), min(h0 + 13, H)
                        nr = r1 - r0
                        ctr = slice(h0 - r0, h0 - r0 + 12)
                        at = pdog.tile([128, 14, W], BF16, tag="at")
                        nc.vector.tensor_add(
                            out=at[:, :nr, 1:95],
                            in0=y[:, r0:r1, 0:94],
                            in1=y[:, r0:r1, 2:96],
                        )
                        nc.vector.tensor_copy(out=at[:, :nr, 0], in_=y[:, r0:r1, 1])
                        nc.vector.tensor_copy(out=at[:, :nr, 95], in_=y[:, r0:r1, 94])
                        pg = {}
                        for gi in (1, 2):
                            pt_ = pdog.tile([128, 14, W], BF16, tag=f"pg{gi}")
                            nc.vector.scalar_tensor_tensor(
                                out=pt_[:, :nr, :],
                                in0=at[:, :nr, :],
                                scalar=cf(cb, gi),
                                in1=y[:, r0:r1, :],
                                op0=AL.mult,
                                op1=AL.add,
                            )
                            pg[gi] = pt_
                        zt = None
                        for gi in (1, 2):
                            pt_ = pg[gi]
                            bt = pdog.tile([128, 12, W], BF16, tag="bt")
                            g0, g1 = max(h0, 1), min(h0 + 12, 95)
                            nc.vector.tensor_add(
                                out=bt[:, g0 - h0 : g1 - h0, :],
                                in0=pt_[:, g0 - 1 - r0 : g1 - 1 - r0, :],
                                in1=pt_[:, g0 + 1 - r0 : g1 + 1 - r0, :],
                            )
                            if h0 == 0:
                                nc.vector.tensor_copy(out=bt[:, 0, :], in_=pt_[:, 1, :])
                            if h0 + 12 == H:
                                nc.vector.tensor_copy(
                                    out=bt[:, 11, :], in_=pt_[:, 94 - r0, :]
                                )
                            gt = pdog.tile([128, 12, W], BF16, tag="gt")
                            nc.vector.scalar_tensor_tensor(
                                out=gt[:],
                                in0=bt[:],
                                scalar=cf(cb, gi),
                                in1=pt_[:, ctr, :],
                                op0=AL.mult,
                                op1=AL.add,
                            )
                            ztn = pdog.tile([128, 12, W], BF16, tag="zt")
                            nc.vector.scalar_tensor_tensor(
                                out=ztn[:],
                                in0=gt[:],
                                scalar=cf(cb, 2 + gi),  # n1 at col 3, -n2 at col 4
                                in1=wcon[:, h0 : h0 + 12, :] if gi == 1 else zt[:],
                                op0=AL.mult,
                                op1=AL.add,
                            )
                            zt = ztn
                        zf = pdog.tile([128, 12, W], BF16, tag="zf")
                        nc.vector.scalar_tensor_tensor(
                            out=zf[:],
                            in0=y[:, h0 : h0 + 12, :],
                            scalar=3.0,
                            in1=zt[:],
                            op0=AL.mult,
                            op1=AL.add,
                        )
                        nc.sync.dma_start(
                            out=zds[cb][:, h0 * W : (h0 + 12) * W], in_=zf[:]
                        )

                # ---- P4: mixer + skip ----
                for ob in range(2):
                    osl = slice(ob * 128, (ob + 1) * 128)
                    of = out_d[s, osl].rearrange("c h w -> c (h w)")
                    kf = skip_d[s, osl].rearrange("c h w -> c (h w)")
                    for ng in range(9):
                        za0 = pp4.tile([128, 1024], BF16, tag="za0")
                        nc.sync.dma_start(out=za0[:], in_=zds[0][:, ts(ng, 1024)])
                        za1 = pp4.tile([128, 1024], BF16, tag="za1")
                        nc.sync.dma_start(out=za1[:], in_=zds[1][:, ts(ng, 1024)])
                        sk = pp4.tile([128, 1024], F32, tag="sk")
                        nc.sync.dma_start(out=sk[:], in_=kf[:, ts(ng, 1024)])
                        ot = pp4.tile([128, 1024], F32, tag="ot")
                        for hh in range(2):
                            pmx = ps_mix.tile([128, 512], F32, tag="pmix")
                            nc.tensor.matmul(
                                pmx[:],
                                lhsT=mw_sb[:, 0 * 2 + ob, :],
                                rhs=za0[:, ts(hh, 512)],
                                start=True,
                                stop=False,
                            )
                            nc.tensor.matmul(
                                pmx[:],
                                lhsT=mw_sb[:, 1 * 2 + ob, :],
                                rhs=za1[:, ts(hh, 512)],
                                start=False,
                                stop=True,
                            )
                            nc.vector.tensor_add(
                                out=ot[:, ts(hh, 512)], in0=pmx[:], in1=sk[:, ts(hh, 512)]
                            )
                        nc.sync.dma_start(out=of[:, ts(ng, 1024)], in_=ot[:])
    nc.finalize()
    return nc


# ---------------- host entry ----------------
def _consts(w_fea, sigma1, sigma2, mixer_w):
    wf = (w_fea.reshape(C).astype(np.float64)) * (2.0 / 3.0)
    tn = []
    for sg in (sigma1, sigma2):
        sig = 2.0 / (1.0 + np.exp(-sg.reshape(C).astype(np.float64)))
        t = np.exp(-1.0 / (2.0 * sig**2))
        tn.append((t, (1.0 + 2.0 * t) ** -2))
    (t1, n1), (t2, n2) = tn
    coef = np.zeros((128, 12), dtype=np.float32)
    for cb in range(2):
        ch = slice(cb * 128, (cb + 1) * 128)
        coef[:, cb * 6 + 0] = wf[ch]
        coef[:, cb * 6 + 1] = t1[ch]
        coef[:, cb * 6 + 2] = t2[ch]
        coef[:, cb * 6 + 3] = n1[ch]
        coef[:, cb * 6 + 4] = -n2[ch]

    aw = np.zeros((96, 3, 96), dtype=np.float64)
    for si, s in enumerate(SCALES):
        ns = int(H * s)
        A = _resize_matrix(ns, H) @ _resize_matrix(H, ns)
        fold = 0.5 if s in (0.25, 0.5) else 1.0
        aw[:, si, :] = (fold * A).T
    aw = aw.astype(ml_dtypes.bfloat16)

    M = mixer_w.reshape(C, C).astype(np.float64)
    mw = np.zeros((128, 4, 128), dtype=np.float64)
    for kc in range(2):
        for ob in range(2):
            mw[:, kc * 2 + ob, :] = M[
                ob * 128 : (ob + 1) * 128, kc * 128 : (kc + 1) * 128
            ].T
    mw = mw.astype(ml_dtypes.bfloat16)
    return aw, mw, coef


def kernel(skip, dec, w_fea, sigma1, sigma2, mixer_w, _trace=[False]):
    skip = np.ascontiguousarray(np.asarray(skip, dtype=np.float32))
    dec = np.ascontiguousarray(np.asarray(dec, dtype=np.float32))
    aw, mw, coef = _consts(
        np.asarray(w_fea), np.asarray(sigma1), np.asarray(sigma2), np.asarray(mixer_w)
    )
    nc = _build()
    in_maps = []
    for i in range(NCORES):
        in_maps.append(
            {
                "dec": dec[BPC * i : BPC * (i + 1)],
                "skip": skip[BPC * i : BPC * (i + 1)],
                "aw": aw,
                "mw": mw,
                "coef": coef,
            }
        )
    res = run_bass_kernel_spmd(nc, in_maps, core_ids=list(range(NCORES)), trace=_trace[0])
    kernel.last_result = res
    return np.concatenate([r["out"] for r in res.results], axis=0)
